# revision 36
# baseline (speedup 1.0000x reference)
"""Trainium2 Bass kernel: 16-head full (non-causal) self-attention with fused
QKV projection, T=4096, E=1024, head_dim=64, tensor-parallel over heads on
8 NeuronCores (2 heads per core).

Design (baseline v1 was ~394us):
  - x is transposed and cast to bf16 on the HOST (input marshaling), so the
    kernel DMAs x^T [E, T] directly: no PE transposes in the warm phase. W is
    likewise host-cast to bf16 and pre-swizzled to the SBUF e-chunk layout.
  - Warm phase: QKV projection only. Q^T/K^T per 512-col chunk (N=512, 8
    accumulating e-chunk matmuls); V per 128-row t-tile (N=128) into V_aug
    (extra ones column per head -> PV also produces softmax row-sums).
  - Attention, software-pipelined S -> exp -> PV with a 3-deep PSUM ring for
    S^T tiles so BOTH exp engines run concurrently and PE matmuls stay
    back-to-back: exp on ScalarE (exact, ~60% of tiles) and VectorE (~40%)
    via a Schraudolph-style bf16 exp — int16(s*(128*log2e/8) + (127*128-8))
    bit-cast as bf16 (max rel err ~4%; fro-level impact ~4e-3, validated
    against reference numerics; total < 1e-2 vs the 2e-2 gate).
  - y is normalized ON DEVICE in y^T layout (reciprocals computed in a
    transposed staging layout via two tiny DMA-xbar transposes, then
    DMA-replicated across partitions) and written out as y^T [h, 64, T];
    the host does the final [T, h, 64] layout transpose (output marshaling,
    mirroring the host-side x^T input marshaling). This keeps the per-tq
    evacuation to ~2 xbar ops so no engine FIFO ever blocks on it.
"""

import numpy as np
import ml_dtypes
from contextlib import ExitStack

import concourse.bass as bass
import concourse.tile as tile
from concourse import bacc, mybir
from concourse.bass import ts
from concourse.bass_utils import run_bass_kernel_spmd

F32 = mybir.dt.float32
BF16 = mybir.dt.bfloat16
I16 = mybir.dt.int16
EXP = mybir.ActivationFunctionType.Exp

T = 4096
E = 1024          # embed dim
HD = 64           # head dim
N_CORES = 8
HPC = 2           # heads per core
WCOLS = 3 * HPC * HD   # 384 W columns per core (q|k|v slices)
VW = HPC * (HD + 1)    # V_aug width per T-tile (2 heads x (64 vals + 1 ones))
SCALE = 1.0 / 8.0      # 1/sqrt(HD)
ECH = E // 128
TT = T // 128
TQ = 512
NTQ = T // TQ
NTK = TT
NPAIR = NTQ * NTK

# Schraudolph bf16 exp constants: bf16(exp(s/8)) ~ bitcast(int16(s*A + B))
SCHR_A = float(np.log2(np.e) * 128.0 / 8.0)
SCHR_B = float(127.0 * 128.0 - 8.0)


def _emit(ctx: ExitStack, tc: "tile.TileContext"):
    nc = tc.nc

    xt_d = nc.dram_tensor("xt", [E, T], BF16, kind="ExternalInput").ap()
    w_d = nc.dram_tensor("w", [128, ECH * WCOLS], BF16, kind="ExternalInput").ap()
    b_d = nc.dram_tensor("b", [WCOLS], F32, kind="ExternalInput").ap()
    # y is produced transposed (head-dim on partitions); host transposes back
    y_d = nc.dram_tensor("y", [HPC, HD, T], F32, kind="ExternalOutput").ap()
    # DRAM bounce buffer for the reciprocal rows (partition replication)
    rs_d = nc.dram_tensor("rscratch", [NTQ, HPC * 4, 128], BF16,
                          kind="Internal").ap()

    const = ctx.enter_context(tc.tile_pool(name="const", bufs=1))
    wb = const.tile([128, ECH * WCOLS], BF16)
    bq = const.tile([128, 1], F32)
    bk = const.tile([128, 1], F32)
    bvb = const.tile([128, HPC * HD], F32)    # b_v broadcast to all partitions
    qT = const.tile([128, T], BF16)
    kT = const.tile([128, T], BF16)
    va = const.tile([128, TT * VW], BF16)
    xTa = const.tile([128, ECH * T], BF16)    # x^T, e-chunk-major

    # ---- weight/bias DMAs (gpsimd queue), x^T stream (sync queue) ----
    nc.gpsimd.dma_start(wb[:], w_d)
    nc.gpsimd.dma_start(bq[:], b_d[0:128])
    nc.gpsimd.dma_start(bk[:], b_d[128:256])
    ones1 = const.tile([1, 128], F32)
    nc.vector.memset(ones1[:], 1.0)
    bvrow = const.tile([1, 128], F32)
    nc.gpsimd.dma_start(bvrow[:], b_d[256:384])
    nc.vector.memset(va[:], 1.0)

    # x^T blocks, ordered so projection can start early; the first chunk's
    # 8 e-blocks are split finer and spread over both DMA queues to cut the
    # pipeline lead-in latency
    for e in range(ECH):
        q = nc.sync if e % 2 == 0 else nc.gpsimd
        q.dma_start(xTa[:, e * T: e * T + 512], xt_d[ts(e, 128), 0:512])
    for e in range(ECH):
        nc.sync.dma_start(xTa[:, e * T + 512: e * T + 1024],
                          xt_d[ts(e, 128), 512:1024])
    for g in range(1, 4):
        for e in range(ECH):
            nc.sync.dma_start(
                xTa[:, e * T + g * 1024: e * T + (g + 1) * 1024],
                xt_d[ts(e, 128), ts(g, 1024)])

    va3 = va.rearrange("p (n two g) -> p n two g", n=TT, two=HPC)

    # ---- warm phase: QKV projection ----
    with tc.tile_pool(name="ps_p", bufs=2, space="PSUM") as ps_p, \
         tc.tile_pool(name="ps_v", bufs=2, space="PSUM") as ps_v, \
         tc.tile_pool(name="ps_m", bufs=1, space="PSUM") as ps_m:
        # broadcast b_v to all partitions via ones-column matmul
        psb = ps_m.tile([128, 128], F32)
        nc.tensor.matmul(psb[:], lhsT=ones1[:], rhs=bvrow[:], start=True, stop=True)
        nc.vector.tensor_copy(bvb[:], psb[:])

        for g in range(NTQ):
            for off, dst, bias in ((0, qT, bq), (128, kT, bk)):
                t = ps_p.tile([128, TQ], F32, tag="psp", name="psp")
                for e in range(ECH):
                    nc.tensor.matmul(
                        t[:], lhsT=wb[:, e * WCOLS + off: e * WCOLS + off + 128],
                        rhs=xTa[:, e * T + g * TQ: e * T + (g + 1) * TQ],
                        start=(e == 0), stop=(e == ECH - 1))
                nc.vector.tensor_scalar_add(dst[:, ts(g, TQ)], t[:], bias[:])
            for tt in range(4 * g, 4 * g + 4):
                psv = ps_v.tile([128, HPC * HD], F32, tag="psv", name="psv")
                for e in range(ECH):
                    nc.tensor.matmul(
                        psv[:], lhsT=xTa[:, e * T + tt * 128: e * T + (tt + 1) * 128],
                        rhs=wb[:, e * WCOLS + 256: (e + 1) * WCOLS],
                        start=(e == 0), stop=(e == ECH - 1))
                nc.vector.tensor_add(
                    va3[:, tt, :, 0:HD],
                    psv[:].rearrange("p (two g) -> p two g", two=HPC),
                    bvb[:].rearrange("p (two g) -> p two g", two=HPC))

    # ---- attention phase: pipelined S -> exp -> PV, staged evac per tq ----
    ps_s = ctx.enter_context(tc.tile_pool(name="ps_s", bufs=3, space="PSUM"))
    ps_y = ctx.enter_context(tc.tile_pool(name="ps_y", bufs=1, space="PSUM"))
    ptp = ctx.enter_context(tc.tile_pool(name="ptp", bufs=8))
    yevp = ctx.enter_context(tc.tile_pool(name="yevp", bufs=3))
    srp = ctx.enter_context(tc.tile_pool(name="srp", bufs=2))
    ytsp = ctx.enter_context(tc.tile_pool(name="ytsp", bufs=2))
    rcsp = ctx.enter_context(tc.tile_pool(name="rcsp", bufs=2))
    rctp = ctx.enter_context(tc.tile_pool(name="rctp", bufs=2))
    rcbp = ctx.enter_context(tc.tile_pool(name="rcbp", bufs=2))
    ynp = ctx.enter_context(tc.tile_pool(name="ynp", bufs=4))

    def emit_S(i):
        tq, tk = divmod(i, NTK)
        pss = ps_s.tile([128, 2 * TQ], F32, tag="pss", name="pss")
        nc.tensor.matmul(pss[:, 0:TQ], lhsT=kT[0:64, ts(tk, 128)],
                         rhs=qT[0:64, ts(tq, TQ)], start=True, stop=True)
        nc.tensor.matmul(pss[:, TQ:2 * TQ], lhsT=kT[64:128, ts(tk, 128)],
                         rhs=qT[64:128, ts(tq, TQ)], start=True, stop=True,
                         tile_position=(64, 0))
        return pss

    def emit_exp(i, pss):
        tq, tk = divmod(i, NTK)
        if tk % 5 in (0, 2):      # 13/32 of tiles on VectorE, never 2-in-a-row
            pt = ptp.tile([128, 2 * TQ], I16, tag="pt", name="ptd")
            nc.vector.tensor_scalar(pt[:], pss[:], SCHR_A, SCHR_B,
                                    mybir.AluOpType.mult, mybir.AluOpType.add)
            return pt[:].bitcast(BF16)
        pt = ptp.tile([128, 2 * TQ], BF16, tag="pt", name="pta")
        nc.scalar.activation(pt[:], pss[:], EXP, scale=SCALE)
        return pt[:]

    def emit_PV(i, psy, ptap):
        tq, tk = divmod(i, NTK)
        for h in range(HPC):
            nc.tensor.matmul(
                psy[h][0:HD + 1, :],
                lhsT=va[:, tk * VW + h * (HD + 1): tk * VW + (h + 1) * (HD + 1)],
                rhs=ptap[:, h * TQ: (h + 1) * TQ],
                start=(tk == 0), stop=(tk == NTK - 1))

    # -- evacuation, split into latency-tolerant stages (fired pairs apart) --
    def evac_a(st):
        # free psy via ScalarE copies; stage both heads' sums rows [8,128]
        # (+pad) for one xbar transpose: yts[q, h*4+c] = sums(chunk c, head h)
        tq, psy = st["tq"], st["psy"]
        yevs = []
        for h in range(HPC):
            yev = yevp.tile([HD + 1, TQ], BF16, tag=f"yev{h}", name=f"yev{h}")
            nc.scalar.copy(yev[:], psy[h][0:HD + 1, :])
            yevs.append(yev)
        st["yevs"] = yevs
        srow = srp.tile([16, 128], BF16, tag="srow", name="srow")
        nc.vector.memset(srow[:], 0.0)
        for h in range(HPC):
            nc.sync.dma_start(
                srow[h * 4:(h + 1) * 4, :],
                yevs[h][HD:HD + 1, :].rearrange("p (c f) -> p c f", c=4))
        yts = ytsp.tile([128, 16], BF16, tag="yts", name="yts")
        nc.sync.dma_start_transpose(yts[:], srow[:])
        st["yts"] = yts
        rcs = rcsp.tile([128, 128], BF16, tag="rcs", name="rcs")
        nc.vector.memset(rcs[:], 0.0)
        st["rcs"] = rcs

    def evac_b(st):
        # reciprocals, per-partition parallel in the transposed layout
        yts, rcs = st["yts"], st["rcs"]
        with nc.allow_low_precision(reason="1/sums in bf16; sums already bf16"):
            for hc in range(HPC * 4):
                nc.vector.reciprocal(rcs[:, hc:hc + 1], yts[:, hc:hc + 1])

    def evac_c(st):
        # transpose recips back to row layout: rct[h*4+c, q] = 1/sums
        rct = rctp.tile([128, 128], BF16, tag="rct", name="rct")
        nc.sync.dma_start_transpose(rct[:], st["rcs"][:])
        st["rct"] = rct

    def evac_c2(st):
        # bounce recip rows to DRAM so they can be read back partition-
        # replicated (SBUF APs cannot have zero partition stride; DRAM can)
        nc.gpsimd.dma_start(rs_d[st["tq"]], st["rct"][0:HPC * 4, :])

    def evac_d(st):
        # replicate the recip rows across HD partitions from DRAM (one DMA)
        rcb = rcbp.tile([HD, HPC * 4 * 128], BF16, tag="rcb", name="rcb")
        tq = st["tq"]
        nc.gpsimd.dma_start(
            rcb[:].rearrange("p (hc f) -> p hc f", hc=HPC * 4),
            rs_d[tq:tq + 1].to_broadcast((HD, HPC * 4, 128)))
        st["rcb"] = rcb

    def evac_e(st):
        # normalize y^T values and DMA out in y^T layout
        tq, yevs, rcb = st["tq"], st["yevs"], st["rcb"]
        for h in range(HPC):
            yn = ynp.tile([HD, TQ], F32, tag="yn", name="yn")
            nc.vector.tensor_mul(yn[:], yevs[h][0:HD, :],
                                 rcb[:, h * TQ:(h + 1) * TQ])
            nc.gpsimd.dma_start(y_d[h, :, tq * TQ:(tq + 1) * TQ], yn[:])

    EVAC_STAGES = ((0, evac_a), (8, evac_b), (12, evac_c), (15, evac_c2),
                   (18, evac_d), (22, evac_e))

    # software pipeline: S runs 3 ahead of PV, exp 3 ahead (ACT/DVE overlap)
    pss_of = {}
    pt_of = {}
    psy = None
    evac_st = None
    LA_S, LA_E = 3, 3
    for j in range(LA_S):
        pss_of[j] = emit_S(j)
    for j in range(LA_E):
        pt_of[j] = emit_exp(j, pss_of[j])
    for i in range(NPAIR):
        tq, tk = divmod(i, NTK)
        if tk == 0:
            prev_psy = psy
            psy = None
        for at_tk, fn in EVAC_STAGES:
            if tk == at_tk and evac_st is not None and evac_st["done"] < at_tk:
                fn(evac_st)
                evac_st["done"] = at_tk
        if tk == 0:
            if prev_psy is not None:
                evac_st = {"tq": tq - 1, "psy": prev_psy, "done": -1}
                evac_a(evac_st)
                evac_st["done"] = 0
            psy = [ps_y.tile([128, TQ], F32, tag=f"psy{h}", name=f"psy{h}")
                   for h in range(HPC)]
        emit_PV(i, psy, pt_of.pop(i))
        del pss_of[i]
        if i + LA_S < NPAIR:
            pss_of[i + LA_S] = emit_S(i + LA_S)
        if i + LA_E < NPAIR:
            pt_of[i + LA_E] = emit_exp(i + LA_E, pss_of[i + LA_E])
    # final tq drain
    last = {"tq": NTQ - 1, "psy": psy, "done": -1}
    for _, fn in EVAC_STAGES:
        fn(last)


def build_program():
    nc = bacc.Bacc("TRN2", target_bir_lowering=False, debug=False,
                   num_devices=N_CORES)
    with tile.TileContext(nc) as tc, ExitStack() as ctx:
        _emit(ctx, tc)
    nc.compile()
    return nc


def shard_inputs(x, W_qkv, b_qkv):
    x = np.asarray(x, dtype=np.float32)
    W = np.asarray(W_qkv, dtype=np.float32)
    b = np.asarray(b_qkv, dtype=np.float32)
    xt = np.ascontiguousarray(x.T).astype(ml_dtypes.bfloat16)
    in_maps = []
    for c in range(N_CORES):
        sl = slice(c * 128, (c + 1) * 128)
        w_c = np.concatenate(
            [W[:, 0 * E:][:, sl], W[:, 1 * E:][:, sl], W[:, 2 * E:][:, sl]],
            axis=1)                                   # [E, 384]
        w_c = np.ascontiguousarray(
            w_c.reshape(ECH, 128, WCOLS).transpose(1, 0, 2).reshape(
                128, ECH * WCOLS)).astype(ml_dtypes.bfloat16)
        b_c = np.concatenate([b[0 * E:][sl], b[1 * E:][sl], b[2 * E:][sl]])
        in_maps.append({"xt": xt, "w": w_c, "b": np.ascontiguousarray(b_c)})
    return in_maps


_PROG = None


def _get_prog():
    global _PROG
    if _PROG is None:
        _PROG = build_program()
    return _PROG


def kernel(x, W_qkv, b_qkv):
    in_maps = shard_inputs(x, W_qkv, b_qkv)
    res = run_bass_kernel_spmd(_get_prog(), in_maps, list(range(N_CORES)))
    y = np.empty((T, 16, HD), np.float32)
    for c in range(N_CORES):
        yt = res.results[c]["y"]                      # [HPC, HD, T]
        for h in range(HPC):
            y[:, HPC * c + h, :] = yt[h].T
    return y
